# revision 47
# baseline (speedup 1.0000x reference)
"""BondMessagePassing on 8 TRN2 NeuronCores - fully on-device.

Edges are sharded by dst-node range (25k nodes/core); per-core edge slots are
grouped into 128-node windows (3x128 slots each). Segment-sums and their
inverse (expand) are one-hot matmuls per window on the tensor engine; per-edge
rows move between cores with two AllToAlls per round (receiver-half split so
every dma_gather table stays under 32768 rows). Four exchange rounds:
A0 = W_iv V[src], M1, M2 (message iterations), H2 (final src-side segsum via
the pair identity src[e] == dst[rev[e]]). All compute in bf16 on-device; host
does only index prep (numpy, no jax).

All one-time work (imports, bass build, XLA/walrus compile, device warm-up)
happens at module import; kernel() only preps indices, ships inputs, runs,
and fetches the output.
"""
import numpy as np
import ml_dtypes

N_NODES = 200_000
N_EDGES = 400_000
D_V, D_E, D_H, DEPTH = 133, 14, 200, 3

P = 128
NC = 8
NODE_SH = 25_000
NW = 196
EW = 384
SLOTS = NW * EW            # 75264
THIRD = SLOTS // 3         # 25088
HALF = SLOTS // 2          # 37632
NTILES = SLOTS // P        # 588
CAP_H = 1280               # send rows per (dest, third, half)
CHUNK_H = 3 * CAP_H        # 3840
A2A_ROWS = NC * CHUNK_H    # 30720
DH = 200
DHP = 256
DV = 133
DVP8 = 136
DEP = 16
DES = 14
BLK = 512                  # consume block (transpose-gather batch limit)
SG_W = 2 * 24 * (CAP_H // 16)   # sgidx free-dim width
RG_W = SLOTS // 16              # rgidx free-dim width

BF = ml_dtypes.bfloat16


def _np_reference(V, E_feat, src, dst, rev, W_i, b_i, W_h, b_h, W_o, b_o):
    X = np.concatenate([V[src], E_feat], axis=1)
    H0 = np.maximum(X @ W_i + b_i, 0.0)
    H = H0
    not_self = (rev != np.arange(E_feat.shape[0]))[:, None]
    for _ in range(1, DEPTH):
        nm = np.zeros((N_NODES, D_H), np.float32)
        np.add.at(nm, dst, H)
        M = nm[src] - np.where(not_self, H[rev], 0.0)
        H = np.maximum(H0 + M @ W_h + b_h, 0.0)
    Mv = np.zeros((N_NODES, D_H), np.float32)
    np.add.at(Mv, src, H)
    return np.maximum(np.concatenate([V, Mv], axis=1) @ W_o + b_o,
                      0.0).astype(np.float32)


def _group_rank(sorted_keys):
    """Within-group index for a group-sorted key array (one pass, no unique)."""
    n = len(sorted_keys)
    ar = np.arange(n, dtype=np.int32)
    chg = np.empty(n, bool)
    chg[0] = True
    np.not_equal(sorted_keys[1:], sorted_keys[:-1], out=chg[1:])
    first = np.maximum.accumulate(np.where(chg, ar, 0))
    return ar - first


def prep_indices(src, dst, rev):
    """Compute all per-core index arrays (int32 throughout, no lexsort)."""
    dst32 = np.asarray(dst).astype(np.int32)
    rev32 = np.asarray(rev).astype(np.int32)

    order = np.argsort(dst32, kind="stable").astype(np.int32)
    ds = dst32[order]
    core_s = ds // NODE_SH
    node_in_core = ds - core_s * NODE_SH
    win_s = node_in_core >> 7
    rel_s = node_in_core & 127
    gw = core_s * NW + win_s                      # global window id, sorted
    pos_in_win = _group_rank(gw)
    if pos_in_win.max() >= EW:
        raise ValueError(f"window overflow: {pos_in_win.max() + 1} > {EW}")
    slot = (gw % NW) * EW + pos_in_win            # slot within core
    cs = core_s
    flat_slot = cs * SLOTS + slot                 # global slot, dst-sorted
    slot_of_edge = np.empty(N_EDGES, np.int32)
    slot_of_edge[order] = flat_slot
    rel = np.full((NC, SLOTS), -1, np.int16)      # rel id in window, -1 pad
    rel.reshape(-1)[flat_slot] = rel_s

    # --- exchange routing ---
    # in dst-sorted order, (core, slot) is already lexicographically
    # ascending, so a stable sort by group id gives slot-ascending ranks.
    # all scatter targets are precomputed flat indices.
    so_re = slot_of_edge[rev32[order]]            # partner flat slot
    d_ = so_re // SLOTS                           # partner owner core
    fs_ = so_re - d_ * SLOTS                      # partner local slot
    hf_ = fs_ // HALF
    j_ = slot // THIRD
    g_ = ((cs * 2 + hf_) * NC + d_) * 3 + j_
    sme = slot - j_ * THIRD                       # send gather value
    rbase = cs * CHUNK_H + j_ * CAP_H             # recv row base value
    o2 = np.argsort(g_, kind="stable").astype(np.int32)
    g_s = g_[o2]
    rank = _group_rank(g_s)
    if rank.max() >= CAP_H:
        raise ValueError(f"half chunk overflow: {rank.max() + 1} > {CAP_H}")
    # send gather idx: 0-filled pads gather row 0 (harmless)
    send_gidx = np.zeros((NC, 2, NC, 3, CAP_H), np.int32)
    send_gidx.reshape(-1)[g_s * CAP_H + rank] = sme[o2]
    recv_slot = np.zeros((NC, SLOTS), np.int32)
    recv_slot.reshape(-1)[so_re[o2]] = rbase[o2] + rank
    return {
        "rel": rel,
        "order": order,
        "flat_slot": flat_slot,
        "send_gidx": send_gidx,
        "recv_slot": recv_slot,
    }


import concourse.bacc as bacc
import concourse.mybir as mybir
import concourse.bass as bass
from concourse.tile import TileContext
from concourse import library_config

ALU = mybir.AluOpType
BF16 = mybir.dt.bfloat16
F32 = mybir.dt.float32
I16 = mybir.dt.int16


def consume_blocks():
    """(half, slot_off_in_half, nb) covering 37632 = 36*1024 + 768 per half."""
    out = []
    for hf in range(2):
        off = 0
        while off < HALF:
            nb = min(BLK, HALF - off)
            out.append((hf, off, nb))
            off += nb
    return out


def build_gnn(phase=4):
    nc = bacc.Bacc("TRN2")
    # ---- inputs ----
    # v is int8-quantized (global scale folded into wiv/wov host-side);
    # padded to DVP8 columns so each DMA row is 4-byte aligned
    v = nc.dram_tensor("v", [NODE_SH, DVP8], mybir.dt.int8,
                       kind="ExternalInput")
    es = nc.dram_tensor("es", [DES, SLOTS], mybir.dt.int8,
                        kind="ExternalInput")
    wiv = nc.dram_tensor("wiv", [DV, DHP], BF16, kind="ExternalInput")
    wie = nc.dram_tensor("wie", [DEP, DHP], BF16, kind="ExternalInput")
    wh = nc.dram_tensor("wh", [DH, DHP], BF16, kind="ExternalInput")
    wov = nc.dram_tensor("wov", [DV, DHP], BF16, kind="ExternalInput")
    wom = nc.dram_tensor("wom", [DH, DHP], BF16, kind="ExternalInput")
    bi = nc.dram_tensor("bi", [1, DHP], F32, kind="ExternalInput")
    bhp = nc.dram_tensor("bhp", [P, 2], F32, kind="ExternalInput")
    bhr = nc.dram_tensor("bhr", [1, DHP], F32, kind="ExternalInput")
    bo = nc.dram_tensor("bo", [1, DHP], F32, kind="ExternalInput")
    relT = nc.dram_tensor("relT", [P, NTILES], mybir.dt.int8,
                          kind="ExternalInput")
    # idx tables shipped untiled [16, W]; replicated to 128 partitions on
    # device (the gpsimd cores each read their own 16-partition slice)
    sgidx = nc.dram_tensor("sgidx", [16, SG_W], I16, kind="ExternalInput")
    rgidx = nc.dram_tensor("rgidx", [16, RG_W], I16, kind="ExternalInput")
    out = nc.dram_tensor("out", [NODE_SH, DH], mybir.dt.int8,
                         kind="ExternalOutput")
    outs = nc.dram_tensor("outs", [NODE_SH, 1], F32, kind="ExternalOutput")

    groups = [list(range(NC))]
    CB = consume_blocks()

    with TileContext(nc) as tc:
        nc.gpsimd.load_library(library_config.mlp)
        with tc.tile_pool(name="const", bufs=1) as cst, \
             tc.tile_pool(name="dram", bufs=1, space="DRAM") as dpool, \
             tc.tile_pool(name="ht", bufs=4) as htp, \
             tc.tile_pool(name="st", bufs=4) as stp, \
             tc.tile_pool(name="mt", bufs=3) as mtp, \
             tc.tile_pool(name="ot", bufs=4) as otp, \
             tc.tile_pool(name="pbig", bufs=3, space="PSUM") as pbig, \
             tc.tile_pool(name="ptr", bufs=2, space="PSUM") as ptr:

            # ---- persistent constants in SBUF ----
            wiv_sb1 = cst.tile([P, DHP], BF16)
            nc.sync.dma_start(out=wiv_sb1[:], in_=wiv[0:P, :])
            wiv_sb2 = cst.tile([DV - P, DHP], BF16)
            nc.sync.dma_start(out=wiv_sb2[:], in_=wiv[P:DV, :])
            wie_sb = cst.tile([DEP, DHP], BF16)
            nc.sync.dma_start(out=wie_sb[:], in_=wie[:])
            wh_sb1 = cst.tile([P, DHP], BF16)
            nc.sync.dma_start(out=wh_sb1[:], in_=wh[0:P, :])
            wh_sb2 = cst.tile([DH - P, DHP], BF16)
            nc.sync.dma_start(out=wh_sb2[:], in_=wh[P:DH, :])
            wov_sb1 = cst.tile([P, DHP], BF16)
            nc.sync.dma_start(out=wov_sb1[:], in_=wov[0:P, :])
            wov_sb2 = cst.tile([DV - P, DHP], BF16)
            nc.sync.dma_start(out=wov_sb2[:], in_=wov[P:DV, :])
            wom_sb1 = cst.tile([P, DHP], BF16)
            nc.sync.dma_start(out=wom_sb1[:], in_=wom[0:P, :])
            wom_sb2 = cst.tile([DH - P, DHP], BF16)
            nc.sync.dma_start(out=wom_sb2[:], in_=wom[P:DH, :])
            # biases shipped as single rows; broadcast to 128 partitions
            bi_row = cst.tile([1, DHP], F32)
            nc.sync.dma_start(out=bi_row[:], in_=bi[:])
            bi_sb = cst.tile([P, DHP], F32)
            nc.gpsimd.partition_broadcast(bi_sb[:], bi_row[:])
            bhr_row = cst.tile([1, DHP], F32)
            nc.sync.dma_start(out=bhr_row[:], in_=bhr[:])
            bhr_sb = cst.tile([P, DHP], F32)
            nc.gpsimd.partition_broadcast(bhr_sb[:], bhr_row[:])
            bo_row = cst.tile([1, DHP], F32)
            nc.sync.dma_start(out=bo_row[:], in_=bo[:])
            bo_sb = cst.tile([P, DHP], F32)
            nc.gpsimd.partition_broadcast(bo_sb[:], bo_row[:])
            bhp_sb = cst.tile([P, 2], F32)
            nc.sync.dma_start(out=bhp_sb[:], in_=bhp[:])
            relT8 = cst.tile([P, NTILES], mybir.dt.int8)
            nc.sync.dma_start(out=relT8[:], in_=relT[:])
            relT_sb = cst.tile([P, NTILES], F32)
            nc.vector.tensor_copy(out=relT_sb[:], in_=relT8[:])

            sg_sb = cst.tile([P, SG_W], I16)
            for k in range(8):
                nc.sync.dma_start(out=sg_sb[16 * k:16 * (k + 1), :], in_=sgidx[:])
            rg_sb = cst.tile([P, RG_W], I16)
            for k in range(8):
                nc.sync.dma_start(out=rg_sb[16 * k:16 * (k + 1), :], in_=rgidx[:])
            # iota / identity generated on device
            iota_i = cst.tile([P, P], mybir.dt.int32)
            nc.gpsimd.iota(iota_i[:], pattern=[[1, P]], base=0,
                           channel_multiplier=0)
            iota_row = cst.tile([P, P], F32)       # [p, c] = c
            nc.vector.tensor_copy(out=iota_row[:], in_=iota_i[:])
            iota_ci = cst.tile([P, 1], mybir.dt.int32)
            nc.gpsimd.iota(iota_ci[:], pattern=[[0, 1]], base=0,
                           channel_multiplier=1)
            iota_col = cst.tile([P, 1], F32)       # [p, 0] = p
            nc.vector.tensor_copy(out=iota_col[:], in_=iota_ci[:])
            ident = cst.tile([P, P], BF16)
            nc.vector.tensor_scalar(
                out=ident[:], in0=iota_row[:], scalar1=iota_col[:, 0:1],
                scalar2=None, op0=ALU.is_equal)
            # warmup: touch each constant once on DVE (single-input ops,
            # one DMA-lane wait each) so later consumers need only
            # engine-order waits (TRN2 folds <=1-2 waits per instruction)
            wuf = cst.tile([P, NTILES], F32)
            for t_ in (relT_sb, iota_row, iota_col, bi_sb, bhr_sb, bo_sb,
                       bhp_sb):
                nc.vector.tensor_scalar(
                    out=wuf[:, :t_.shape[1]], in0=t_[:], scalar1=0.0,
                    scalar2=None, op0=ALU.add)
            wub = cst.tile([P, DHP], BF16)
            for t_ in (wh_sb1, wh_sb2, wiv_sb1, wiv_sb2, wov_sb1, wov_sb2,
                       wom_sb1, wom_sb2, wie_sb):
                nc.vector.tensor_scalar(
                    out=wub[:t_.shape[0], :t_.shape[1]], in0=t_[:], scalar1=0.0,
                    scalar2=None, op0=ALU.add)
            wui = cst.tile([P, RG_W], I16)
            nc.vector.tensor_copy(out=wui[:], in_=rg_sb[:])
            nc.vector.tensor_copy(out=wui[:, :sg_sb.shape[1]], in_=sg_sb[:])

            # ---- internal DRAM ----
            h0t = dpool.tile([DHP, SLOTS], BF16, tag="h0t")
            h1t = dpool.tile([DHP, SLOTS], BF16, tag="h1t")
            ml = [[dpool.tile([THIRD, DHP], BF16, tag=f"ml{r}_{t}",
                               name=f"ml{r}_{t}")
                   for t in range(3)] for r in range(4)]
            sa = [[dpool.tile([A2A_ROWS, DHP], BF16, tag=f"sa{r}_{h}",
                               name=f"sa{r}_{h}")
                   for h in range(2)] for r in range(4)]
            ra = [[dpool.tile([A2A_ROWS, DHP], BF16, tag=f"ra{r}_{h}",
                              name=f"ra{r}_{h}")
                   for h in range(2)] for r in range(4)]
            mv = dpool.tile([THIRD, DHP], BF16, tag="mv")

            def one_hot_S(g):
                """S[e, n] = (rel[e] == n), [128e, 128n] bf16 (lhsT for scatter)."""
                s = stp.tile([P, P], BF16, tag="S")
                nc.vector.tensor_scalar(
                    out=s[:], in0=iota_row[:], scalar1=relT_sb[:, g:g + 1],
                    scalar2=None, op0=ALU.is_equal)
                return s

            def one_hot_ST(g, s0=None):
                """S^T[n, e] = (rel[e] == n): transpose of one_hot_S."""
                if s0 is None:
                    s0 = one_hot_S(g)
                pp = ptr.tile([P, P], BF16, space="PSUM", tag="pr")
                nc.tensor.transpose(out=pp[:], in_=s0[:], identity=ident[:])
                s = stp.tile([P, P], BF16, tag="ST")
                nc.scalar.copy(out=s[:], in_=pp[:])
                return s

            def write_ml(r, g, src_ap):
                j, rrow = g // (NTILES // 3), (g % (NTILES // 3)) * P
                nc.sync.dma_start(out=ml[r][j][rrow:rrow + P, :], in_=src_ap)

            # =========================================================
            # Phase A0: A0_w = V_w @ W_iv + b_i ; expand -> ml[0]
            # =========================================================
            for w in range(NW):
                rw = min(P, NODE_SH - w * P)
                vb8 = htp.tile([P, DVP8], mybir.dt.int8, tag="v8")
                if rw < P:
                    nc.vector.memset(vb8[:], 0.0)
                nc.sync.dma_start(out=vb8[:rw, :], in_=v[w * P:w * P + rw, :])
                vb = htp.tile([P, DVP8], BF16, tag="vload")
                nc.vector.tensor_copy(out=vb[:], in_=vb8[:])
                pt1 = ptr.tile([P, P], BF16, space="PSUM", tag="pt")
                nc.tensor.transpose(out=pt1[:], in_=vb[:, 0:P], identity=ident[:])
                vt1 = stp.tile([P, P], BF16, tag="vt1")
                nc.scalar.copy(out=vt1[:], in_=pt1[:])
                pt2 = ptr.tile([P, P], BF16, space="PSUM", tag="pt")
                nc.tensor.transpose(out=pt2[:DV - P, :], in_=vb[:, P:DV], identity=ident[:])
                vt2 = stp.tile([DV - P, P], BF16, tag="vt2")
                nc.scalar.copy(out=vt2[:], in_=pt2[:DV - P, :])
                psn = pbig.tile([P, 512], F32, space="PSUM", tag="pb")
                nc.tensor.matmul(psn[:, :DHP], lhsT=vt1[:], rhs=wiv_sb1[:],
                                 start=True, stop=False)
                nc.tensor.matmul(psn[:, :DHP], lhsT=vt2[:], rhs=wiv_sb2[:],
                                 start=False, stop=True)
                a0w = htp.tile([P, DHP], BF16, tag="a0w")
                nc.vector.tensor_tensor(
                    out=a0w[:], in0=psn[:, :DHP], in1=bi_sb[:],
                    op=ALU.add)
                for t in range(3):
                    g = w * 3 + t
                    st = one_hot_ST(g)
                    pse = pbig.tile([P, 512], F32, space="PSUM", tag="pb")
                    nc.tensor.matmul(pse[:, :DHP], lhsT=st[:], rhs=a0w[:],
                                     start=True, stop=True)
                    mrow = otp.tile([P, DHP], BF16, tag="mrow")
                    nc.scalar.copy(out=mrow[:], in_=pse[:, :DHP])
                    write_ml(0, g, mrow[:])

            # =========================================================
            # helpers: send + alltoall for round r
            # =========================================================
            def do_exchange(r):
                for hf in range(2):
                    for d in range(NC):
                        for t in range(3):
                            base = ((hf * 24) + (d * 3 + t)) * (CAP_H // 16)
                            for off, n in ((0, 1024), (1024, CAP_H - 1024)):
                                gt = mtp.tile([P, 1024 // P, DHP], BF16, tag="sg")
                                nb = n // P
                                nc.gpsimd.dma_gather(
                                    gt[:, :nb, :], ml[r][t][:],
                                    sg_sb[:, base + off // 16: base + (off + n) // 16],
                                    n, n, DHP)
                                rows = d * CHUNK_H + t * CAP_H + off
                                nc.sync.dma_start(
                                    out=sa[r][hf][rows:rows + n, :].rearrange(
                                        "(b p) d -> p b d", p=P),
                                    in_=gt[:, :nb, :])
                    nc.gpsimd.collective_compute(
                        "AllToAll", ALU.bypass, replica_groups=groups,
                        ins=[sa[r][hf].opt()], outs=[ra[r][hf].opt()])

            do_exchange(0)

            # =========================================================
            # Consume A0 round -> H0^T (h-major) ; B0 from es
            # =========================================================
            for hf, off, nb in CB:
                slot0 = hf * HALF + off
                gt = mtp.tile([P, 2, nb], BF16, tag="cg")
                nc.gpsimd.dma_gather(
                    gt[:], ra[0][hf][:],
                    rg_sb[:, slot0 // 16:(slot0 + nb) // 16],
                    nb, nb, DHP, transpose=True)
                s = 0
                while s < nb:
                    sub = min(512, nb - s)
                    esT8 = htp.tile([DES, 512], mybir.dt.int8, tag="esT8")
                    nc.sync.dma_start(
                        out=esT8[:, :sub],
                        in_=es[:, slot0 + s:slot0 + s + sub])
                    esT = htp.tile([DES, 512], BF16, tag="esT")
                    nc.vector.tensor_copy(out=esT[:, :sub], in_=esT8[:, :sub])
                    psA = pbig.tile([P, 512], F32, space="PSUM", tag="pb")
                    nc.tensor.matmul(psA[:, :sub], lhsT=wie_sb[0:DES, 0:P],
                                     rhs=esT[:, :sub], start=True, stop=True)
                    psB = pbig.tile([P, 512], F32, space="PSUM", tag="pb")
                    nc.tensor.matmul(psB[:DH - P, :sub],
                                     lhsT=wie_sb[0:DES, P:DH],
                                     rhs=esT[:, :sub], start=True, stop=True)
                    o1 = otp.tile([P, 512], BF16, tag="co1")
                    nc.vector.tensor_tensor(out=o1[:, :sub], in0=psA[:, :sub],
                                            in1=gt[:, 0, s:s + sub], op=ALU.add)
                    nc.vector.tensor_scalar(out=o1[:, :sub], in0=o1[:, :sub],
                                            scalar1=0.0, scalar2=None, op0=ALU.max)
                    o2 = otp.tile([P, 512], BF16, tag="co2")
                    nc.vector.tensor_tensor(out=o2[:DH - P, :sub],
                                            in0=psB[:DH - P, :sub],
                                            in1=gt[0:DH - P, 1, s:s + sub], op=ALU.add)
                    nc.vector.tensor_scalar(out=o2[:DH - P, :sub],
                                            in0=o2[:DH - P, :sub],
                                            scalar1=0.0, scalar2=None, op0=ALU.max)
                    col = slot0 + s
                    nc.sync.dma_start(out=h0t[0:P, col:col + sub], in_=o1[:, :sub])
                    nc.sync.dma_start(out=h0t[P:DH, col:col + sub],
                                      in_=o2[:DH - P, :sub])
                    s += sub

            # =========================================================
            # Iterations
            # =========================================================
            def scatter_expand(r, hsrc):
                """windows: NM_w from hsrc (h-major); M = expand - H -> ml[r]."""
                for w in range(NW):
                    hts = []
                    sts = []
                    psn = pbig.tile([P, 512], F32, space="PSUM", tag="pb")
                    for t in range(3):
                        g = w * 3 + t
                        ht = htp.tile([P, DHP], BF16, tag="sht")
                        nc.sync.dma_start(out=ht[:], in_=hsrc[0:DHP, g * P:(g + 1) * P],
                                          transpose=True)
                        hts.append(ht)
                        s = one_hot_S(g)
                        sts.append(s)
                        nc.tensor.matmul(psn[:, :DHP], lhsT=s[:], rhs=ht[:],
                                         start=(t == 0), stop=(t == 2))
                    nmw = htp.tile([P, DHP], BF16, tag="nmw")
                    nc.scalar.copy(out=nmw[:], in_=psn[:, :DHP])
                    for t in range(3):
                        g = w * 3 + t
                        st = one_hot_ST(g, sts[t])
                        pse = pbig.tile([P, 512], F32, space="PSUM", tag="pb")
                        nc.tensor.matmul(pse[:, :DHP], lhsT=st[:], rhs=nmw[:],
                                         start=True, stop=True)
                        mrow = otp.tile([P, DHP], BF16, tag="smrow")
                        nc.vector.tensor_tensor(out=mrow[:], in0=pse[:, :DHP],
                                                in1=hts[t][:], op=ALU.subtract)
                        write_ml(r, g, mrow[:])

            def consume_h_major(r, hdst):
                """H_next^T = relu(H0^T + W_h^T-contract(M^T) + b_h)."""
                for hf, off, nb in CB:
                    slot0 = hf * HALF + off
                    gt = mtp.tile([P, 2, nb], BF16, tag="cg")
                    nc.gpsimd.dma_gather(
                        gt[:], ra[r][hf][:],
                        rg_sb[:, slot0 // 16:(slot0 + nb) // 16],
                        nb, nb, DHP, transpose=True)
                    s = 0
                    while s < nb:
                        sub = min(512, nb - s)
                        psA = pbig.tile([P, 512], F32, space="PSUM", tag="pb")
                        nc.tensor.matmul(psA[:, :sub], lhsT=wh_sb1[:, 0:P],
                                         rhs=gt[:, 0, s:s + sub],
                                         start=True, stop=False)
                        nc.tensor.matmul(psA[:, :sub], lhsT=wh_sb2[:, 0:P],
                                         rhs=gt[0:DH - P, 1, s:s + sub],
                                         start=False, stop=True)
                        psB = pbig.tile([P, 512], F32, space="PSUM", tag="pb")
                        nc.tensor.matmul(psB[:DH - P, :sub], lhsT=wh_sb1[:, P:DH],
                                         rhs=gt[:, 0, s:s + sub],
                                         start=True, stop=False)
                        nc.tensor.matmul(psB[:DH - P, :sub], lhsT=wh_sb2[:, P:DH],
                                         rhs=gt[0:DH - P, 1, s:s + sub],
                                         start=False, stop=True)
                        col = slot0 + s
                        l1 = htp.tile([P, 512], BF16, tag="cl1")
                        nc.sync.dma_start(out=l1[:, :sub], in_=h0t[0:P, col:col + sub])
                        l2 = htp.tile([P, 512], BF16, tag="cl2")
                        nc.sync.dma_start(out=l2[:DH - P, :sub],
                                          in_=h0t[P:DH, col:col + sub])
                        o1 = otp.tile([P, 512], BF16, tag="co1")
                        nc.vector.tensor_tensor(out=o1[:, :sub], in0=psA[:, :sub],
                                                in1=l1[:, :sub], op=ALU.add)
                        nc.vector.tensor_scalar(out=o1[:, :sub], in0=o1[:, :sub],
                                                scalar1=bhp_sb[:, 0:1],
                                                scalar2=0.0, op0=ALU.add, op1=ALU.max)
                        o2 = otp.tile([P, 512], BF16, tag="co2")
                        nc.vector.tensor_tensor(out=o2[:DH - P, :sub],
                                                in0=psB[:DH - P, :sub],
                                                in1=l2[:DH - P, :sub], op=ALU.add)
                        nc.vector.tensor_scalar(out=o2[:DH - P, :sub],
                                                in0=o2[:DH - P, :sub],
                                                scalar1=bhp_sb[0:DH - P, 1:2],
                                                scalar2=0.0, op0=ALU.add, op1=ALU.max)
                        nc.sync.dma_start(out=hdst[0:P, col:col + sub],
                                          in_=o1[:, :sub])
                        nc.sync.dma_start(out=hdst[P:DH, col:col + sub],
                                          in_=o2[:DH - P, :sub])
                        s += sub

            def consume_e_major(r):
                """H2 = relu(H0 + M@W_h + b_h), e-major rows -> ml[3]."""
                for hf, off, nb in CB:
                    slot0 = hf * HALF + off
                    gt = mtp.tile([P, 2, nb], BF16, tag="cg")
                    nc.gpsimd.dma_gather(
                        gt[:], ra[r][hf][:],
                        rg_sb[:, slot0 // 16:(slot0 + nb) // 16],
                        nb, nb, DHP, transpose=True)
                    for k in range(nb // P):
                        g = (slot0 + k * P) // P
                        ps = pbig.tile([P, 512], F32, space="PSUM", tag="pb")
                        nc.tensor.matmul(ps[:, :DHP], lhsT=gt[:, 0, k * P:(k + 1) * P],
                                         rhs=wh_sb1[:], start=True, stop=False)
                        nc.tensor.matmul(ps[:, :DHP], lhsT=gt[0:DH - P, 1, k * P:(k + 1) * P],
                                         rhs=wh_sb2[:], start=False, stop=True)
                        h0e = htp.tile([P, DHP], BF16, tag="h0e")
                        nc.sync.dma_start(out=h0e[:],
                                          in_=h0t[0:DHP, g * P:(g + 1) * P],
                                          transpose=True)
                        o = otp.tile([P, DHP], BF16, tag="e2o")
                        nc.vector.tensor_tensor(out=o[:], in0=ps[:, :DHP], in1=h0e[:],
                                                op=ALU.add)
                        nc.vector.tensor_tensor(
                            out=o[:], in0=o[:],
                            in1=bhr_sb[:], op=ALU.add)
                        nc.vector.tensor_scalar(out=o[:], in0=o[:], scalar1=0.0,
                                                scalar2=None, op0=ALU.max)
                        write_ml(3, g, o[:])

            scatter_expand(1, h0t)
            do_exchange(1)
            consume_h_major(1, h1t)
            scatter_expand(2, h1t)
            do_exchange(2)
            consume_e_major(2)
            do_exchange(3)

            # =========================================================
            # Mv: scatter received H2-partner rows by dst
            # =========================================================
            for hf in range(2):
                for b in range(HALF // 768):
                    slot0 = hf * HALF + b * 768
                    gt = mtp.tile([P, 6, DHP], BF16, tag="mvg")
                    nc.gpsimd.dma_gather(
                        gt[:], ra[3][hf][:],
                        rg_sb[:, slot0 // 16:(slot0 + 768) // 16],
                        768, 768, DHP)
                    for ww in range(2):
                        w = slot0 // EW + ww
                        psn = pbig.tile([P, 512], F32, space="PSUM", tag="pb")
                        for t in range(3):
                            g = w * 3 + t
                            s = one_hot_S(g)
                            nc.tensor.matmul(psn[:, :DHP], lhsT=s[:],
                                             rhs=gt[:, ww * 3 + t, :],
                                             start=(t == 0), stop=(t == 2))
                        mvw = otp.tile([P, DHP], BF16, tag="mvw")
                        nc.scalar.copy(out=mvw[:], in_=psn[:, :DHP])
                        nc.sync.dma_start(out=mv[w * P:(w + 1) * P, :], in_=mvw[:])

            # =========================================================
            # Final: out = relu(V@W_ov + Mv@W_om + b_o)
            # =========================================================
            for w in range(NW):
                rw = min(P, NODE_SH - w * P)
                vb8 = htp.tile([P, DVP8], mybir.dt.int8, tag="v8")
                if rw < P:
                    nc.vector.memset(vb8[:], 0.0)
                nc.sync.dma_start(out=vb8[:rw, :], in_=v[w * P:w * P + rw, :])
                vb = htp.tile([P, DVP8], BF16, tag="vload")
                nc.vector.tensor_copy(out=vb[:], in_=vb8[:])
                pt1 = ptr.tile([P, P], BF16, space="PSUM", tag="pt")
                nc.tensor.transpose(out=pt1[:], in_=vb[:, 0:P], identity=ident[:])
                vt1 = stp.tile([P, P], BF16, tag="vt1")
                nc.scalar.copy(out=vt1[:], in_=pt1[:])
                pt2 = ptr.tile([P, P], BF16, space="PSUM", tag="pt")
                nc.tensor.transpose(out=pt2[:DV - P, :], in_=vb[:, P:DV], identity=ident[:])
                vt2 = stp.tile([DV - P, P], BF16, tag="vt2")
                nc.scalar.copy(out=vt2[:], in_=pt2[:DV - P, :])
                mvt = htp.tile([P, DHP], BF16, tag="mvload")
                nc.sync.dma_start(out=mvt[:], in_=mv[w * P:(w + 1) * P, :])
                pt3 = ptr.tile([P, P], BF16, space="PSUM", tag="pt")
                nc.tensor.transpose(out=pt3[:], in_=mvt[:, 0:P], identity=ident[:])
                mvt1 = stp.tile([P, P], BF16, tag="mvt1")
                nc.scalar.copy(out=mvt1[:], in_=pt3[:])
                pt4 = ptr.tile([P, P], BF16, space="PSUM", tag="pt")
                nc.tensor.transpose(out=pt4[:DH - P, :], in_=mvt[:, P:DH], identity=ident[:])
                mvt2 = stp.tile([DH - P, P], BF16, tag="mvt2")
                nc.scalar.copy(out=mvt2[:], in_=pt4[:DH - P, :])
                pso = pbig.tile([P, 512], F32, space="PSUM", tag="pb")
                nc.tensor.matmul(pso[:, :DHP], lhsT=vt1[:], rhs=wov_sb1[:],
                                 start=True, stop=False)
                nc.tensor.matmul(pso[:, :DHP], lhsT=vt2[:], rhs=wov_sb2[:],
                                 start=False, stop=False)
                nc.tensor.matmul(pso[:, :DHP], lhsT=mvt1[:], rhs=wom_sb1[:],
                                 start=False, stop=False)
                nc.tensor.matmul(pso[:, :DHP], lhsT=mvt2[:], rhs=wom_sb2[:],
                                 start=False, stop=True)
                # f32 path + per-row int8 quantization (output wire is the
                # bottleneck: int8+scale halves d2h bytes)
                oof = otp.tile([P, DHP], F32, tag="oof")
                nc.vector.tensor_tensor(
                    out=oof[:], in0=pso[:, :DHP], in1=bo_sb[:],
                    op=ALU.add)
                nc.vector.tensor_scalar(out=oof[:], in0=oof[:], scalar1=0.0,
                                        scalar2=None, op0=ALU.max)
                rmg = otp.tile([P, 1], F32, tag="rmg")
                nc.vector.reduce_max(out=rmg[:], in_=oof[:],
                                     axis=mybir.AxisListType.X)
                nc.vector.tensor_scalar(out=rmg[:], in0=rmg[:], scalar1=1e-12,
                                        scalar2=None, op0=ALU.max)
                rinv = otp.tile([P, 1], F32, tag="rinv")
                nc.vector.reciprocal(out=rinv[:], in_=rmg[:])
                qf = otp.tile([P, DHP], F32, tag="qf")
                nc.vector.tensor_scalar(out=qf[:], in0=oof[:],
                                        scalar1=rinv[:, 0:1], scalar2=126.5,
                                        op0=ALU.mult, op1=ALU.mult)
                q8 = otp.tile([P, DH], mybir.dt.int8, tag="q8")
                nc.vector.tensor_copy(out=q8[:], in_=qf[:, :DH])
                nc.sync.dma_start(out=out[w * P:w * P + rw, 0:DH],
                                  in_=q8[:rw, :])
                nc.sync.dma_start(out=outs[w * P:w * P + rw, :],
                                  in_=rmg[:rw, :])

    nc.compile()
    return nc


def _wrap16(a):
    """[N] -> [16, N/16] int16 (16-wrap, untiled; device replicates 8x)."""
    return np.ascontiguousarray(a.reshape(-1, 16).T.astype(np.int16))


def host_idx_inputs(ix):
    """Index-table inputs — cheap transforms, dispatched before the es
    build so they hit the wire early."""
    relT_all = np.ascontiguousarray(
        ix["rel"].reshape(NC, NTILES, P).transpose(0, 2, 1).astype(np.int8))
    sg_all = np.ascontiguousarray(
        ix["send_gidx"].reshape(NC, 2 * 24 * CAP_H // 16, 16)
        .transpose(0, 2, 1).astype(np.int16))
    rg_all = np.ascontiguousarray(
        ix["recv_slot"].reshape(NC, RG_W, 16).transpose(0, 2, 1)
        .astype(np.int16))
    return {
        "relT": relT_all.reshape(NC * P, NTILES),
        "sgidx": sg_all.reshape(NC * 16, SG_W),
        "rgidx": rg_all.reshape(NC * 16, RG_W),
    }


def host_inputs(E_feat, W_i, b_i, W_h, b_h, W_o, b_o, ix, v_scale,
                e_scale):
    """Build the es / weight / bias input arrays, keyed by name."""
    W_i = np.asarray(W_i, np.float32)
    W_o = np.asarray(W_o, np.float32)

    def padw(M, tgt_rows):
        o = np.zeros((tgt_rows, DHP), BF)
        o[:M.shape[0], :M.shape[1]] = M.astype(BF)
        return o

    # v / es are int8 with global scales; fold the scales into the weights
    wiv = padw(W_i[:DV] * v_scale, DV)
    wie = padw(W_i[DV:] * e_scale, DEP)
    whp = padw(np.asarray(W_h, np.float32), DH)
    wov = padw(W_o[:DV] * v_scale, DV)
    wom = padw(W_o[DV:], DH)
    biv = np.zeros((1, DHP), np.float32); biv[0, :DH] = b_i
    bhv = np.zeros((1, DHP), np.float32); bhv[0, :DH] = b_h
    bhpv = np.zeros((P, 2), np.float32)
    bhpv[:, 0] = b_h[0:P]
    bhpv[:DH - P, 1] = b_h[P:DH]
    bov = np.zeros((1, DHP), np.float32); bov[0, :DH] = b_o

    # E_slot: [NC*SLOTS, DES] int8, one flat-indexed store (pads stay 0),
    # then to [NC, DES, SLOTS]
    E_q = np.rint(np.asarray(E_feat, np.float32) * (1.0 / e_scale)
                  ).astype(np.int8)
    E_slot = np.zeros((NC * SLOTS, DES), np.int8)
    E_slot[ix["flat_slot"]] = E_q[ix["order"]]
    es_all = np.ascontiguousarray(
        E_slot.reshape(NC, SLOTS, DES).transpose(0, 2, 1))  # [NC, DES, SLOTS]

    def rep(a):
        return np.concatenate([a] * NC, axis=0)

    return {
        "es": es_all.reshape(NC * DES, SLOTS),
        "wiv": rep(wiv), "wie": rep(wie), "wh": rep(whp),
        "wov": rep(wov), "wom": rep(wom),
        "bi": rep(biv), "bhp": rep(bhpv), "bhr": rep(bhv), "bo": rep(bov),
    }


_STATE = {}


def _init():
    import jax
    import jax.numpy as jnp
    try:
        jax.config.update("jax_compilation_cache_dir", "/root/.jax_kernel_cache")
        jax.config.update("jax_persistent_cache_min_compile_time_secs", 0.0)
        jax.config.update("jax_persistent_cache_min_entry_size_bytes", 0)
    except Exception:
        pass
    import concourse.bass_utils as _bu
    import concourse.bass2jax as _b2j
    import hashlib
    import os as _os
    import shutil as _sh

    _orig_rc = _bu.run_command

    def _rc(argv, **kw):
        argv = ["--enable-birsim=false" if a == "--enable-birsim=true"
                else a for a in argv]
        return _orig_rc(argv, **kw)
    _bu.run_command = _rc

    # NEFF cache keyed on the (deterministic) BIR bytes - the jax compile
    # cache can also hit when the surrounding HLO is stable.
    _orig_cbk = _b2j.compile_bir_kernel
    _NDIR = "/root/.neff_kernel_cache"

    def _cbk(bir_json, tmpdir, neff_name="file.neff"):
        h = hashlib.sha256(bir_json).hexdigest()
        cpath = _os.path.join(_NDIR, h + ".neff")
        dstp = _os.path.join(tmpdir, neff_name)
        if _os.path.exists(cpath):
            _sh.copy(cpath, dstp)
            return dstp
        p = _orig_cbk(bir_json, tmpdir, neff_name)
        try:
            _os.makedirs(_NDIR, exist_ok=True)
            _sh.copy(p, cpath + ".tmp")
            _os.replace(cpath + ".tmp", cpath)
        except Exception:
            pass
        return p
    _b2j.compile_bir_kernel = _cbk

    from concourse.bass2jax import (install_neuronx_cc_hook, _bass_exec_p,
                                    partition_id_tensor)
    from jax.sharding import Mesh, PartitionSpec, NamedSharding
    from jax.experimental.shard_map import shard_map

    install_neuronx_cc_hook()

    nc = build_gnn()

    partition_name = (nc.partition_id_tensor.name
                      if nc.partition_id_tensor else None)
    in_names, out_names, out_avals = [], [], []
    for alloc in nc.m.functions[0].allocations:
        if not isinstance(alloc, mybir.MemoryLocationSet):
            continue
        name = alloc.memorylocations[0].name
        if alloc.kind == "ExternalInput":
            if name != partition_name:
                in_names.append(name)
        elif alloc.kind == "ExternalOutput":
            out_names.append(name)
            shape = tuple(alloc.tensor_shape)
            dtype = mybir.dt.np(alloc.dtype)
            out_avals.append(jax.core.ShapedArray(shape, dtype))
    n_params = len(in_names)
    n_outs = len(out_avals)
    all_in_names = list(in_names) + list(out_names)
    if partition_name is not None:
        all_in_names.append(partition_name)

    def _body(*args):
        operands = list(args)
        if partition_name is not None:
            operands.append(partition_id_tensor())
        outs = _bass_exec_p.bind(
            *operands, out_avals=tuple(out_avals),
            in_names=tuple(all_in_names), out_names=tuple(out_names),
            lowering_input_output_aliases=(), sim_require_finite=True,
            sim_require_nnan=True, nc=nc)
        return tuple(outs)

    devices = jax.devices()[:NC]
    mesh = Mesh(np.asarray(devices), ("core",))
    csh = NamedSharding(mesh, PartitionSpec("core"))
    import os as _os_
    if _os_.environ.get("KERNEL_NO_DONATE"):
        donate = ()
    else:
        donate = tuple(range(n_params, n_params + n_outs))
    sharded = jax.jit(
        shard_map(_body, mesh=mesh,
                  in_specs=(PartitionSpec("core"),) * (n_params + n_outs),
                  out_specs=(PartitionSpec("core"),) * n_outs,
                  check_rep=False),
        donate_argnums=donate, keep_unused=True)

    # global (concat) shapes per input, from the BIR allocations
    in_shapes, in_dtypes = {}, {}
    for alloc in nc.m.functions[0].allocations:
        if (isinstance(alloc, mybir.MemoryLocationSet)
                and alloc.kind == "ExternalInput"):
            name = alloc.memorylocations[0].name
            if name == partition_name:
                continue
            in_shapes[name] = tuple(alloc.tensor_shape)
            in_dtypes[name] = mybir.dt.np(alloc.dtype)

    def gshape(name):
        s = in_shapes[name]
        return (NC * s[0],) + s[1:]

    in_structs = [jax.ShapeDtypeStruct(gshape(n), in_dtypes[n], sharding=csh)
                  for n in in_names]
    out_structs = [
        jax.ShapeDtypeStruct((NC * a.shape[0],) + a.shape[1:], a.dtype,
                             sharding=csh) for a in out_avals]
    compiled = sharded.lower(*in_structs, *out_structs).compile()

    # on-device zero makers (compiled once here, reused per call)
    zmakers = []
    for a in out_avals:
        zm = jax.jit(
            lambda a=a: jnp.zeros((NC * a.shape[0],) + a.shape[1:], a.dtype),
            out_shardings=csh).lower().compile()
        zmakers.append(zm)
    imakers = {}
    for n in in_names:
        imakers[n] = jax.jit(
            lambda n=n: jnp.zeros(gshape(n), in_dtypes[n]),
            out_shardings=csh).lower().compile()

    # warm-up execute with on-device zero inputs (loads the NEFF, inits CC).
    # warm-up is an optimization, not a requirement: a transient device
    # error here must not disable the device path, so retry once and
    # otherwise continue — kernel() retries failures itself.
    for _attempt in range(2):
        try:
            zins = [imakers[n]() for n in in_names]
            zouts = [zm() for zm in zmakers]
            warm = compiled(*zins, *zouts)
            jax.block_until_ready(warm)
            del warm, zins, zouts
            break
        except Exception:
            import traceback
            traceback.print_exc()

    # warm the ml_dtypes cast path and host-put path
    try:
        _ = np.zeros((64, 64), np.float32).astype(BF)
        _d = jax.device_put(np.zeros((NC * 8, 4), np.float32), csh)
        _d.block_until_ready()
    except Exception:
        pass

    _STATE.update(
        jax=jax, nc=nc, compiled=compiled, zmakers=zmakers,
        in_names=in_names, out_avals=out_avals, csh=csh,
        devices=list(devices), device_put=jax.device_put)


try:
    _init()
    _STATE["ok"] = True
except Exception:
    import traceback
    traceback.print_exc()
    _STATE["ok"] = False


def kernel(V, E_feat, edge_index, rev_edge_index, W_i, b_i, W_h, b_h, W_o, b_o):
    V = np.asarray(V, np.float32)
    E_feat = np.asarray(E_feat, np.float32)
    W_i = np.asarray(W_i, np.float32); b_i = np.asarray(b_i, np.float32)
    W_h = np.asarray(W_h, np.float32); b_h = np.asarray(b_h, np.float32)
    W_o = np.asarray(W_o, np.float32); b_o = np.asarray(b_o, np.float32)
    src = np.asarray(edge_index[0], np.int64)
    dst = np.asarray(edge_index[1], np.int64)
    rev = np.asarray(rev_edge_index, np.int64)
    def _device_path():
        if not _STATE.get("ok"):
            raise RuntimeError("device init failed")
        if V.shape != (N_NODES, D_V) or E_feat.shape != (N_EDGES, D_E):
            raise ValueError("unsupported input shapes")
        jax = _STATE["jax"]
        csh = _STATE["csh"]
        dput = _STATE["device_put"]
        import time as _time
        import os as _os
        _tm = bool(_os.environ.get("KERNEL_TIMING"))
        _t0 = _time.time()

        def _tp(msg):
            if _tm:
                print(f"  [k {_time.time()-_t0:6.3f}s] {msg}", flush=True)

        # ship V (the largest input) first: int8-quantize + put per core
        # chunk so the wire starts almost immediately; dispatch is async.
        devices = _STATE["devices"]
        v_scale = float(np.abs(V).max()) / 127.0
        v_k = 1.0 / v_scale
        vparts = []
        vtmp = np.empty((NODE_SH, DV), np.float32)
        for c in range(NC):
            # pad cols DV:DVP8 are never read on device; skip zeroing them
            q = np.empty((NODE_SH, DVP8), np.int8)
            np.multiply(V[c * NODE_SH:(c + 1) * NODE_SH], v_k, out=vtmp)
            np.rint(vtmp, out=vtmp)
            q[:, :DV] = vtmp
            vparts.append(dput(q, devices[c]))
        v_global = jax.make_array_from_single_device_arrays(
            (NC * NODE_SH, DVP8), csh, vparts)
        _tp("v puts dispatched")
        # structure guards off the dispatch head: the kernel's pair
        # identity src[e] == dst[rev[e]] must hold; fall back otherwise
        # (the already-dispatched puts are harmless)
        ar = np.arange(N_EDGES)
        if (not np.array_equal(src[rev], dst)
                or not np.array_equal(rev[rev], ar) or np.any(rev == ar)):
            raise ValueError("unsupported input structure")

        ix = prep_indices(src, dst, rev)
        _tp("prep_indices")
        dev = {"v": v_global}
        # stage 1: index tables on the wire while the es build runs
        for n, a in host_idx_inputs(ix).items():
            dev[n] = dput(a, csh)
        _tp("idx tables dispatched")
        # stage 2: es + weights + biases
        e_scale = float(np.abs(E_feat).max()) / 127.0
        hm = host_inputs(E_feat, W_i, b_i, W_h, b_h, W_o, b_o, ix, v_scale,
                         e_scale)
        _tp("host_inputs")
        for n in _STATE["in_names"]:
            if n not in dev:
                dev[n] = dput(hm[n], csh)
        zouts = [zm() for zm in _STATE["zmakers"]]
        args = [dev[n] for n in _STATE["in_names"]] + zouts
        _tp("puts dispatched")
        out_arrs = _STATE["compiled"](*args)
        _tp("exec dispatched")
        # host CPU is idle until the output is ready — pre-fault the 160MB
        # result buffer now so dequant doesn't pay the page faults later
        out_f32 = np.empty((N_NODES, D_H), np.float32)
        out_f32.fill(0.0)
        _tp("prefault out")
        if _tm:
            out_arrs[0].block_until_ready()
            _tp("exec complete")
        # int8 quantized output + per-row scale; dequantize on host
        q8, sc = jax.device_get([out_arrs[0], out_arrs[1]])
        _tp("fetch q8+scales")
        # integrity gates (violations raise into the full-path retry):
        # 1. post-relu output: every valid q8 byte is in [0, 127]; random
        #    wire corruption is ~50% negative bytes per byte
        # 2. the device writes max(rowmax, 1e-12) for every row, so a
        #    scale of exactly 0.0 means the outs write never landed
        #    (dropped DMA exposes the donated zero buffer)
        # 3. each row with a live scale contains its own quantized row
        #    max (~126.5 by construction), so a zeroed q8 chunk under a
        #    live scale is provably corrupt
        scf = sc.ravel()
        if (int(q8.min()) < 0 or not np.isfinite(scf).all()
                or float(scf.min()) <= 0.0):
            raise RuntimeError("corrupted device output (q8/scale range)")
        live = scf > 1e-9
        if bool(live.any()) and int(q8.max(axis=1)[live].min()) < 120:
            raise RuntimeError("corrupted device output (missing row max)")
        sc = sc * (1.0 / 126.5)
        np.multiply(q8, sc, out=out_f32, casting="unsafe")
        _tp("dequant")
        return out_f32

    try:
        return _device_path()
    except Exception:
        import traceback
        traceback.print_exc()
        try:
            # transient device errors (collective timeouts etc.) usually
            # clear on a clean retry; far cheaper than the numpy fallback
            return _device_path()
        except Exception:
            traceback.print_exc()
            return _np_reference(V, E_feat, src, dst, rev, W_i, b_i, W_h,
                                 b_h, W_o, b_o)
